# revision 40
# baseline (speedup 1.0000x reference)
"""Multi-head attention layer on 8 Trainium2 NeuronCores.

Reference (per batch n):
    Q = x@Wq + bq; K = x@Wk + bk; V = x@Wv + bv       (16 heads, Dh=64)
    out = softmax(Q K^T / sqrt(Dh)) V  -> concat heads -> @Wo + bo

Sharding: 2 head-groups (tensor parallel) x 4 batches (data parallel) = 8
cores. Core c handles batch c%4 and heads [8*(c//4), 8*(c//4)+8). Each core
computes a partial output projection with its Wo row-block; the host sums
the two head-group partials per batch (the only cross-core reduction).

Per-core kernel (bf16 matmul inputs, fp32 PSUM accumulation). The exp
chain on ScalarE (~294us at 1 elem/cycle/lane) is the critical resource;
everything else is scheduled under it:
  - K^T/Q^T in PAIR layout [128, 4, seq]: partitions 0:64 = even head's
    d_head, 64:128 = odd head's. The QK^T scores for the two heads of a
    pair run CONCURRENTLY as 64x128 row-tiles of the PE array
    (tile_position (0,0)/(64,0)), halving score-matmul time vs a padded
    128-row contraction. V in [seq, d_head] layout with an appended ones
    column (softmax denominators fall out of the PV matmul for free).
  - slot pipeline over (mc, pair, st): each slot's two score matmuls fill
    one [128,1024] PSUM tile (512 cols/head); ONE ScalarE exp per slot
    (max-width ACT amortizes its ~352-cycle fixed cost); two PV matmuls
    accumulate per-head O^T (+denominator row). PSUM: spt ping-pong 4
    banks + 2 op accumulators + 1 projection + 1 norm broadcast = 8.
  - projections/out-projection dribble one matmul at a time into PE slack
    between attention matmuls (deadline-ordered FIFO filler queue);
    normalization (broadcast-reciprocal matmul + DVE mult) is deferred
    off the critical path.

Self-contained: hardcodes shapes for x:[4,2048,1024], d_model=1024,
16 heads; a no-bias graph variant is compiled when all biases are zero.
"""

import sys
import types

import numpy as np

import concourse.mybir as mybir
import concourse.tile as tile
from concourse import bacc
from concourse.bass_utils import run_bass_kernel_spmd

f32 = mybir.dt.float32
f32r = mybir.dt.float32r
bf16 = mybir.dt.bfloat16
AF = mybir.ActivationFunctionType
N_CORES = 8
P = 128

# ---------------------------------------------------------------------------


def build_nc(L=2048, D=1024, HPC=8, Dh=64, WB=True):
    """Build the per-core Bass graph (SPMD: same graph, per-core shards)."""
    KO = D // P          # k-tiles over d_model
    DQ = HPC * Dh        # local projected dim (512)
    NP = HPC // 2        # head pairs (4)
    ST = L // P          # 128-row seq (kpos) tiles (16)
    MC = L // 512        # 512-wide query chunks (4)
    NSC = L // 512       # 512-wide seq chunks for projections (4)
    WKO = DQ // P        # k-tiles for out-proj contraction (4)
    EC = D // 512        # 512-wide out chunks (2)
    MS = L // P          # 128-row out row-tiles (16)

    nc = bacc.Bacc("TRN2", target_bir_lowering=False, debug=False,
                   num_devices=N_CORES)

    # host-packed layouts: each partition reads one contiguous line per DMA
    xT_d = nc.dram_tensor("xTc", [NSC * P, KO * 512], bf16,
                          kind="ExternalInput")
    Wq_d = nc.dram_tensor("Wqc", [NP * P, KO * P], bf16,
                          kind="ExternalInput")
    Wk_d = nc.dram_tensor("Wkc", [NP * P, KO * P], bf16,
                          kind="ExternalInput")
    Wv_d = nc.dram_tensor("Wvc", [P, KO * DQ], bf16, kind="ExternalInput")
    Wo_d = nc.dram_tensor("Woc", [P, WKO * D], bf16, kind="ExternalInput")
    bq_d = nc.dram_tensor("bq", [DQ], bf16, kind="ExternalInput")
    bk_d = nc.dram_tensor("bk", [DQ], bf16, kind="ExternalInput")
    bv_d = nc.dram_tensor("bv", [DQ], bf16, kind="ExternalInput")
    bo_d = nc.dram_tensor("bo", [D], bf16, kind="ExternalInput")
    out_d = nc.dram_tensor("out", [L, D], f32, kind="ExternalOutput")

    xT_v = xT_d.ap().rearrange("(sc p) (ko s) -> sc p ko s", p=P, ko=KO)
    Wq_v = Wq_d.ap().rearrange("(np p) (ko c) -> np p ko c", p=P, ko=KO)
    Wk_v = Wk_d.ap().rearrange("(np p) (ko c) -> np p ko c", p=P, ko=KO)
    Wv_v = Wv_d.ap().rearrange("p (ko d) -> p ko d", ko=KO)
    Wo_v = Wo_d.ap().rearrange("p (ko e) -> p ko e", ko=WKO)
    out_v = out_d.ap().rearrange("(ms p) e -> p ms e", p=P)

    with tile.TileContext(nc) as tc:
        with (
            tc.tile_pool(name="pp", bufs=1) as pp,
            tc.tile_pool(name="wp", bufs=1) as wp,
            tc.tile_pool(name="sp", bufs=1) as sp,
            tc.tile_pool(name="ps", bufs=1, space="PSUM") as ps,
        ):
            # ---- persistent tiles ----
            KTP = pp.tile([P, NP, L], bf16, name="KTP")
            QTP = pp.tile([P, NP, L], bf16, name="QTP")
            VA = pp.tile([P, ST, HPC, Dh + 1], bf16, name="VA")
            OT = pp.tile([P, WKO, L], bf16, name="OT")
            ones_f = pp.tile([P, P], f32, name="ones_f")
            ones_r = pp.tile([P, P], f32r, name="ones_r")
            ones_b = pp.tile([1, 512], bf16, name="ones_b")
            nc.vector.memset(ones_f[:], 1.0)
            nc.vector.tensor_copy(ones_r[:], ones_f[:])
            nc.vector.memset(ones_b[:], 1.0)
            nc.vector.tensor_copy(VA[:, :, :, Dh:Dh + 1],
                                  ones_f[:, 0:1].to_broadcast((P, ST, HPC, 1)))
            if WB:
                bqs = pp.tile([1, DQ], bf16, name="bqs")
                bks = pp.tile([1, DQ], bf16, name="bks")
                bvs = pp.tile([1, DQ], bf16, name="bvs")
                bos = pp.tile([1, D], bf16, name="bos")
                nc.sync.dma_start(bqs[:], bq_d.ap()[None, :])
                nc.sync.dma_start(bks[:], bk_d.ap()[None, :])
                nc.sync.dma_start(bvs[:], bv_d.ap()[None, :])
                nc.sync.dma_start(bos[:], bo_d.ap()[None, :])

            xts_tiles = [None] * NSC

            def issue_xts_dma(sc):
                xts = sp.tile([P, KO, 512], bf16, tag="xts", bufs=NSC,
                              name=f"xts{sc}")
                nc.sync.dma_start(xts[:], xT_v[sc])
                xts_tiles[sc] = xts

            # ---- projection chains (single-matmul generator steps) ----
            def v_steps(st):
                """V projection for one 128-row seq tile -> VA[:, st]."""
                sc, ssub = st // 4, st % 4
                pv = ps.tile([P, 512], f32, tag="proj", bufs=2,
                             name=f"pv{st}")
                for ko in range(KO):
                    nc.tensor.matmul(
                        pv[:, 0:DQ],
                        lhsT=xts_tiles[sc][:, ko, ssub * P:(ssub + 1) * P],
                        rhs=Wv_sb[:, ko, :],
                        start=(ko == 0), stop=(not WB and ko == KO - 1))
                    yield
                if WB:
                    nc.tensor.matmul(pv[:, 0:DQ], lhsT=ones_b[0:1, 0:P],
                                     rhs=bvs[0:1, :], start=False, stop=True)
                nc.vector.tensor_copy(
                    VA[:, st, :, 0:Dh],
                    pv[:, 0:DQ].rearrange("p (h d) -> p h d", d=Dh))
                yield

            def kt_steps(p, sc, wt_cell):
                """K^T projection: KTP[:, p, sc*512:(sc+1)*512]."""
                pt = ps.tile([P, 512], f32, tag="proj", bufs=2,
                             name=f"pk{sc}_{p}")
                for ko in range(KO):
                    nc.tensor.matmul(pt[:], lhsT=wt_cell[0][:, ko, :],
                                     rhs=xts_tiles[sc][:, ko, :],
                                     start=(ko == 0),
                                     stop=(not WB and ko == KO - 1))
                    yield
                if WB:
                    nc.tensor.matmul(
                        pt[:], lhsT=bks[0:1, p * P:(p + 1) * P],
                        rhs=ones_b[0:1, 0:512], start=False, stop=True)
                nc.vector.tensor_copy(KTP[:, p, sc * 512:(sc + 1) * 512],
                                      pt[:])
                yield

            def qt_steps(p, mc, wt_cell):
                """Q^T projection: QTP[:, p, mc*512:(mc+1)*512]."""
                pt = ps.tile([P, 512], f32, tag="proj", bufs=2,
                             name=f"pq{mc}_{p}")
                for ko in range(KO):
                    nc.tensor.matmul(pt[:], lhsT=wt_cell[0][:, ko, :],
                                     rhs=xts_tiles[mc][:, ko, :],
                                     start=(ko == 0),
                                     stop=(not WB and ko == KO - 1))
                    yield
                if WB:
                    nc.tensor.matmul(
                        pt[:], lhsT=bqs[0:1, p * P:(p + 1) * P],
                        rhs=ones_b[0:1, 0:512], start=False, stop=True)
                nc.vector.tensor_copy(QTP[:, p, mc * 512:(mc + 1) * 512],
                                      pt[:])
                yield

            def wk_prep(p, wt_cell):
                wt = sp.tile([P, KO, P], bf16, tag="wk", bufs=4,
                             name=f"wk{p}")
                nc.sync.dma_start(wt[:], Wk_v[p])
                wt_cell[0] = wt

            def wq_prep(p, wt_cell):
                wt = sp.tile([P, KO, P], bf16, tag="wq", bufs=4,
                             name=f"wq{p}")
                nc.sync.dma_start(wt[:], Wq_v[p])
                wt_cell[0] = wt

            # ---- deferred softmax normalization ----
            pending = []

            def emit_norm_tail(item):
                """Broadcast-reciprocal matmul + normalize into OT."""
                dnr, ot, h, mc = item
                bp = ps.tile([P, 512], f32, tag="proj", bufs=2,
                             name=f"bp{h}_{mc}")
                nc.tensor.matmul(bp[0:Dh, :], lhsT=ones_r[0:1, 0:Dh],
                                 rhs=dnr[0:1, :], start=True, stop=True)
                half = Dh * (h % 2)
                nc.vector.tensor_tensor(
                    OT[half:half + Dh, h // 2, mc * 512:(mc + 1) * 512],
                    ot[:], bp[0:Dh, :], mybir.AluOpType.mult)

            def flush_norms(mcm, max_pair=NP - 1):
                due = [it for it in pending
                       if it[3] == mcm and it[2] // 2 <= max_pair]
                for it in due:
                    pending.remove(it)
                    emit_norm_tail(it)

            def outproj_steps(ms, Wo_sb):
                """Full out-projection chain for one 128-row tile."""
                mcm = (ms * P) // 512
                flush_norms(mcm)
                for ec in range(EC):
                    pt = ps.tile([P, 512], f32, tag="proj", bufs=2,
                                 name=f"po{ms}_{ec}")
                    for ko in range(WKO):
                        nc.tensor.matmul(
                            pt[:], lhsT=OT[:, ko, ms * P:(ms + 1) * P],
                            rhs=Wo_sb[:, ko, ec * 512:(ec + 1) * 512],
                            start=(ko == 0),
                            stop=(not WB and ko == WKO - 1))
                        yield
                    if WB:
                        nc.tensor.matmul(pt[:], lhsT=ones_b[0:1, 0:P],
                                         rhs=bos[0:1,
                                                 ec * 512:(ec + 1) * 512],
                                         start=False, stop=True)
                    os_ = sp.tile([P, 512], f32, tag="os", bufs=3,
                                  name=f"os{ms}_{ec}")
                    nc.vector.tensor_copy(os_[:], pt[:])
                    nc.sync.dma_start(out_v[:, ms, ec * 512:(ec + 1) * 512],
                                      os_[:])
                    yield

            # ko-split out-projection for the LAST mc: pairs {0,1} partial
            # accumulates early to SBUF; tail only runs pairs {2,3} + add.
            osacc = {}

            def opj_partial_steps(ms, Wo_sb):
                mcm = (ms * P) // 512
                flush_norms(mcm, max_pair=1)
                for ec in range(EC):
                    pt = ps.tile([P, 512], f32, tag="proj", bufs=2,
                                 name=f"pp{ms}_{ec}")
                    for ko in range(2):
                        nc.tensor.matmul(
                            pt[:], lhsT=OT[:, ko, ms * P:(ms + 1) * P],
                            rhs=Wo_sb[:, ko, ec * 512:(ec + 1) * 512],
                            start=(ko == 0), stop=(ko == 1))
                        yield
                    acc = sp.tile([P, 512], f32, tag="oacc", bufs=8,
                                  name=f"oacc{ms}_{ec}")
                    nc.vector.tensor_copy(acc[:], pt[:])
                    osacc[(ms, ec)] = acc
                    yield

            def opj_final_steps(ms, Wo_sb):
                mcm = (ms * P) // 512
                flush_norms(mcm)
                for ec in range(EC):
                    pt = ps.tile([P, 512], f32, tag="proj", bufs=2,
                                 name=f"pf{ms}_{ec}")
                    for ko in range(2, WKO):
                        nc.tensor.matmul(
                            pt[:], lhsT=OT[:, ko, ms * P:(ms + 1) * P],
                            rhs=Wo_sb[:, ko, ec * 512:(ec + 1) * 512],
                            start=(ko == 2),
                            stop=(not WB and ko == WKO - 1))
                        yield
                    if WB:
                        nc.tensor.matmul(pt[:], lhsT=ones_b[0:1, 0:P],
                                         rhs=bos[0:1,
                                                 ec * 512:(ec + 1) * 512],
                                         start=False, stop=True)
                    os_ = sp.tile([P, 512], f32, tag="os", bufs=3,
                                  name=f"osf{ms}_{ec}")
                    nc.vector.tensor_tensor(os_[:], pt[:],
                                            osacc[(ms, ec)][:],
                                            mybir.AluOpType.add)
                    nc.sync.dma_start(out_v[:, ms, ec * 512:(ec + 1) * 512],
                                      os_[:])
                    yield

            # ---- FIFO filler queue: [avail, deadline, gen, prep] ----
            fq = []

            def prefetch(k):
                """Issue weight DMAs for the next few queued chains."""
                for ent in fq[:3]:
                    if ent[3] is not None:
                        ent[3]()
                        ent[3] = None

            def drain_overdue(k):
                while fq and fq[0][1] <= k:
                    ent = fq.pop(0)
                    if ent[3] is not None:
                        ent[3]()
                    for _ in ent[2]:
                        pass

            def filler_step(k):
                if fq and fq[0][0] <= k:
                    if fq[0][3] is not None:
                        fq[0][3]()
                        fq[0][3] = None
                    try:
                        next(fq[0][2])
                    except StopIteration:
                        fq.pop(0)
                        filler_step(k)

            # ---- attention slot pipeline --------------------------------
            # staggered block order: each pair's first block (its K/Q
            # deadline) arrives progressively, and each mc column finishes
            # evenly spaced so out-projection dribbles instead of piling
            # into the final phase.
            blocks = [(0, 0), (1, 0), (0, 1), (2, 0), (1, 1), (3, 0),
                      (2, 1), (0, 2), (3, 1), (1, 2), (2, 2), (0, 3),
                      (3, 2), (1, 3), (2, 3), (3, 3)]
            bidx = {b: i for i, b in enumerate(blocks)}
            slots = [(mc, p, st) for p, mc in blocks for st in range(ST)]
            NS = len(slots)
            ops = {}

            def emit_S(k):
                """Row-tiled score pair + the slot's single exp ACT."""
                mc, p, st = slots[k]
                drain_overdue(k)
                spt = ps.tile([P, 1024], f32, tag="spt", bufs=2,
                              name=f"spt{k}")
                ksl = slice(st * P, (st + 1) * P)
                qsl = slice(mc * 512, (mc + 1) * 512)
                nc.tensor.matmul(spt[:, 0:512], lhsT=KTP[0:64, p, ksl],
                                 rhs=QTP[0:64, p, qsl],
                                 start=True, stop=True, tile_position=(0, 0))
                nc.tensor.matmul(spt[:, 512:1024], lhsT=KTP[64:128, p, ksl],
                                 rhs=QTP[64:128, p, qsl],
                                 start=True, stop=True, tile_position=(64, 0))
                es = sp.tile([P, 1024], bf16, tag="es", bufs=5,
                             name=f"es{k}")
                nc.scalar.activation(es[:], spt[:], AF.Exp, scale=0.125)
                return es

            def emit_PV(k, es):
                mc, p, st = slots[k]
                if st == 0:
                    ops[(mc, p)] = [
                        ps.tile([P, 512], f32, tag="op", bufs=2,
                                name=f"op{mc}_{p}_{i}") for i in range(2)]
                opA, opB = ops[(mc, p)]
                nc.tensor.matmul(opA[0:Dh + 1, :], lhsT=VA[:, st, 2 * p, :],
                                 rhs=es[:, 0:512],
                                 start=(st == 0), stop=(st == ST - 1))
                nc.tensor.matmul(opB[0:Dh + 1, :],
                                 lhsT=VA[:, st, 2 * p + 1, :],
                                 rhs=es[:, 512:1024],
                                 start=(st == 0), stop=(st == ST - 1))
                if st == ST - 1:
                    block_end(mc, p)

            def block_end(mc, p):
                """Copy O^T + denominators out of PSUM, queue normalization."""
                pair_ops = ops.pop((mc, p))
                dns, ots = [], []
                for i in range(2):
                    op = pair_ops[i]
                    dn = sp.tile([1, 512], f32, tag="dn", bufs=6,
                                 name=f"dn{mc}_{p}_{i}")
                    nc.vector.tensor_copy(dn[:], op[Dh:Dh + 1, :])
                    ot = sp.tile([Dh, 512], f32, tag="ott", bufs=4,
                                 name=f"ot{mc}_{p}_{i}")
                    nc.vector.tensor_copy(ot[:], op[0:Dh, :])
                    dns.append(dn)
                    ots.append(ot)
                for i in range(2):
                    dn, ot = dns[i], ots[i]
                    nc.vector.reciprocal_approx_fast(dn[:], dn[:])
                    dnr = sp.tile([1, 512], f32r, tag="dnr", bufs=6,
                                  name=f"dnr{mc}_{p}_{i}")
                    nc.vector.tensor_copy(dnr[:], dn[:])
                    pending.append((dnr, ot, 2 * p + i, mc))

            # ---- prologue: DMAs ordered for earliest first score ----
            issue_xts_dma(0)
            wk_cells = {p: [None] for p in range(NP)}
            wq_cells = {p: [None] for p in range(NP)}
            wk_prep(0, wk_cells[0])
            wq_prep(0, wq_cells[0])
            Wv_sb = wp.tile([P, KO, DQ], bf16, name="Wv_sb")
            nc.sync.dma_start(Wv_sb[:], Wv_v)
            for sc in range(1, NSC):
                issue_xts_dma(sc)
            for p in range(1, NP):
                wk_prep(p, wk_cells[p])
                wq_prep(p, wq_cells[p])
            Wo_sb = wp.tile([P, WKO, D], bf16, name="Wo_sb")
            nc.sync.dma_start(Wo_sb[:], Wo_v)
            for _ in kt_steps(0, 0, wk_cells[0]):
                pass
            for _ in qt_steps(0, 0, wq_cells[0]):
                pass

            # ---- build filler queue (sorted by deadline; margins so each
            # chain's Vector CAST lands before its consumer slot) ----
            first_blk = {p: min(bidx[(p, mc)] for mc in range(MC))
                         for p in range(NP)}
            ents = []
            for st in range(ST):
                ents.append([0, st + 1, v_steps(st), None])
            for sc in range(1, NSC):
                ents.append([0, max(1, 4 * sc - 6),
                             kt_steps(0, sc, wk_cells[0]), None])
            for mc in range(1, MC):
                ents.append([0, max(1, 16 * bidx[(0, mc)] - 6),
                             qt_steps(0, mc, wq_cells[0]), None])
            for p in range(1, NP):
                for sc in range(NSC):
                    ents.append([0, 16 * first_blk[p] + 4 * sc - 6,
                                 kt_steps(p, sc, wk_cells[p]), None])
                for mc in range(MC):
                    ents.append([0, 16 * bidx[(p, mc)] - 6,
                                 qt_steps(p, mc, wq_cells[p]), None])
            last_mc = blocks[-1][1]
            for ms in range(MS):
                mcm = (ms * P) // 512
                if mcm == last_mc:
                    avail = 16 * (bidx[(1, mcm)] + 1) + 2 + 2 * (ms % 4)
                    ents.append([avail, min(NS, avail + 30),
                                 opj_partial_steps(ms, Wo_sb), None])
                    ents.append([NS, NS, opj_final_steps(ms, Wo_sb), None])
                else:
                    avail = (16 * (bidx[(NP - 1, mcm)] + 1) + 2
                             + 2 * (ms % 4))
                    ents.append([avail, min(NS, avail + 40),
                                 outproj_steps(ms, Wo_sb), None])
            ents.sort(key=lambda e: (e[1], e[0]))
            fq.extend(ents)

            # ---- main pipeline (PV lags its slot by 2 for jitter slack;
            # fillers go FIRST so they run while ACT drains the spt the
            # next S is waiting on — never behind a blocked PV) ----
            es_live = {0: emit_S(0), 1: emit_S(1)}
            for k in range(NS):
                st = slots[k][2]
                if st in (6, 11) and pending:
                    emit_norm_tail(pending.pop(0))
                else:
                    filler_step(k)
                    filler_step(k)
                    if k < 80 or k >= 112:
                        filler_step(k)
                if k + 2 < NS:
                    es_live[k + 2] = emit_S(k + 2)
                emit_PV(k, es_live.pop(k))

            # ---- tail ----
            while pending:
                emit_norm_tail(pending.pop(0))
            while fq:
                ent = fq.pop(0)
                if ent[3] is not None:
                    ent[3]()
                for _ in ent[2]:
                    pass

    nc.compile()
    return nc


# ---------------------------------------------------------------------------

_NC_CACHE = {}


def _get_nc(with_biases=True):
    key = ("nc", with_biases)
    if key not in _NC_CACHE:
        _NC_CACHE[key] = build_nc(WB=with_biases)
    return _NC_CACHE[key]


def _install_ntff_hook():
    """Provide antenv.axon_hooks (absent in this image) so trace=True can
    capture NTFF profiles for timing."""
    if "antenv.axon_hooks" in sys.modules:
        return
    mod = types.ModuleType("antenv.axon_hooks")
    holder = [None]
    mod.set_axon_ntff_profile_hook = lambda hk: holder.__setitem__(0, hk)
    mod.get_axon_ntff_profile_hook = lambda: holder[0]
    sys.modules["antenv.axon_hooks"] = mod
    import antenv

    antenv.axon_hooks = mod
    try:
        from trn_agent_boot.trn_boot import _ntff_profile_via_ctypes

        mod.set_axon_ntff_profile_hook(
            _ntff_profile_via_ctypes("/opt/axon/libaxon_pjrt.so"))
    except Exception:
        pass


def _make_in_maps(x, Wq, bq, Wk, bk, Wv, bv, Wo, bo):
    import ml_dtypes

    NB, L, D = x.shape          # 4, 2048, 1024
    DQ = D // 2                 # head-group width (8 heads x 64)
    P_, KO, NSC, NP, WKO = 128, D // 128, L // 512, DQ // 128, DQ // 128

    def pack_x(xn):             # [L,D] -> [NSC*128, KO*512]
        return (xn.T.reshape(KO, P_, NSC, 512).transpose(2, 1, 0, 3)
                .reshape(NSC * P_, KO * 512))

    def pack_w(w):              # [D, DQ] -> [NP*128, KO*128]
        return (w.reshape(KO, P_, NP, P_).transpose(2, 1, 0, 3)
                .reshape(NP * P_, KO * P_))

    def pack_rows(w, ko, cols):  # [ko*128, cols] -> [128, ko*cols]
        return (w.reshape(ko, P_, cols).transpose(1, 0, 2)
                .reshape(P_, ko * cols))

    in_maps = []
    for c in range(N_CORES):
        n, g = c % 4, c // 4
        sl = slice(g * DQ, (g + 1) * DQ)
        bf = ml_dtypes.bfloat16
        in_maps.append({
            "xTc": np.ascontiguousarray(pack_x(x[n])).astype(bf),
            "Wqc": np.ascontiguousarray(pack_w(Wq[:, sl])).astype(bf),
            "Wkc": np.ascontiguousarray(pack_w(Wk[:, sl])).astype(bf),
            "Wvc": np.ascontiguousarray(
                pack_rows(Wv[:, sl], KO, DQ)).astype(bf),
            "Woc": np.ascontiguousarray(
                pack_rows(Wo[sl, :], WKO, D)).astype(bf),
            "bq": np.ascontiguousarray(bq[sl]).astype(bf),
            "bk": np.ascontiguousarray(bk[sl]).astype(bf),
            "bv": np.ascontiguousarray(bv[sl]).astype(bf),
            "bo": (bo if g == 0 else np.zeros_like(bo)).astype(bf),
        })
    return in_maps


def run_sharded(inputs, trace=False):
    """Run the SPMD kernel on the full inputs. Returns (output, exec_ns)."""
    wb = any(
        np.asarray(inputs[k]).any() for k in ("bq", "bk", "bv", "bo"))
    nc = _get_nc(with_biases=bool(wb))
    if trace:
        _install_ntff_hook()
    in_maps = _make_in_maps(**inputs)
    res = run_bass_kernel_spmd(nc, in_maps, list(range(N_CORES)), trace=trace)
    outs = [res.results[c]["out"] for c in range(N_CORES)]
    full = np.stack([outs[n] + outs[n + 4] for n in range(4)], axis=0)
    return full.astype(np.float32), res.exec_time_ns


def kernel(**inputs):
    out, _ = run_sharded(inputs, trace=False)
    return out


# revision 41
# speedup vs baseline: 1.1863x; 1.1863x over previous
"""Multi-head attention layer on 8 Trainium2 NeuronCores.

Reference (per batch n):
    Q = x@Wq + bq; K = x@Wk + bk; V = x@Wv + bv       (16 heads, Dh=64)
    out = softmax(Q K^T / sqrt(Dh)) V  -> concat heads -> @Wo + bo

Sharding: 2 head-groups (tensor parallel) x 4 batches (data parallel) = 8
cores. Core c handles batch c%4 and heads [8*(c//4), 8*(c//4)+8). Each core
computes a partial output projection with its Wo row-block; the host sums
the two head-group partials per batch (the only cross-core reduction).

Per-core kernel (bf16 matmul inputs, fp32 PSUM accumulation). The exp
chain on ScalarE (~294us at 1 elem/cycle/lane) is the critical resource;
everything else is scheduled under it:
  - K^T/Q^T in PAIR layout [128, 4, seq]: partitions 0:64 = even head's
    d_head, 64:128 = odd head's. The QK^T scores for the two heads of a
    pair run CONCURRENTLY as 64x128 row-tiles of the PE array
    (tile_position (0,0)/(64,0)), halving score-matmul time vs a padded
    128-row contraction. V in [seq, d_head] layout with an appended ones
    column (softmax denominators fall out of the PV matmul for free).
  - slot pipeline over (mc, pair, st): each slot's two score matmuls fill
    one [128,1024] PSUM tile (512 cols/head); ONE ScalarE exp per slot
    (max-width ACT amortizes its ~352-cycle fixed cost); two PV matmuls
    accumulate per-head O^T (+denominator row). PSUM: spt ping-pong 4
    banks + 2 op accumulators + 1 projection + 1 norm broadcast = 8.
  - projections/out-projection dribble one matmul at a time into PE slack
    between attention matmuls (deadline-ordered FIFO filler queue);
    normalization (broadcast-reciprocal matmul + DVE mult) is deferred
    off the critical path.

Self-contained: hardcodes shapes for x:[4,2048,1024], d_model=1024,
16 heads; a no-bias graph variant is compiled when all biases are zero.
"""

import sys
import types

import numpy as np

import concourse.mybir as mybir
import concourse.tile as tile
from concourse import bacc
from concourse.bass_utils import run_bass_kernel_spmd

f32 = mybir.dt.float32
f32r = mybir.dt.float32r
bf16 = mybir.dt.bfloat16
AF = mybir.ActivationFunctionType
N_CORES = 8
P = 128

# ---------------------------------------------------------------------------


def build_nc(L=2048, D=1024, HPC=8, Dh=64, WB=True):
    """Build the per-core Bass graph (SPMD: same graph, per-core shards)."""
    KO = D // P          # k-tiles over d_model
    DQ = HPC * Dh        # local projected dim (512)
    NP = HPC // 2        # head pairs (4)
    ST = L // P          # 128-row seq (kpos) tiles (16)
    MC = L // 512        # 512-wide query chunks (4)
    NSC = L // 512       # 512-wide seq chunks for projections (4)
    WKO = DQ // P        # k-tiles for out-proj contraction (4)
    EC = D // 512        # 512-wide out chunks (2)
    MS = L // P          # 128-row out row-tiles (16)

    nc = bacc.Bacc("TRN2", target_bir_lowering=False, debug=False,
                   num_devices=N_CORES)

    # host-packed layouts: each partition reads one contiguous line per DMA
    xT_d = nc.dram_tensor("xTc", [NSC * P, KO * 512], bf16,
                          kind="ExternalInput")
    Wq_d = nc.dram_tensor("Wqc", [NP * P, KO * P], bf16,
                          kind="ExternalInput")
    Wk_d = nc.dram_tensor("Wkc", [NP * P, KO * P], bf16,
                          kind="ExternalInput")
    Wv_d = nc.dram_tensor("Wvc", [P, KO * DQ], bf16, kind="ExternalInput")
    Wo_d = nc.dram_tensor("Woc", [P, WKO * D], bf16, kind="ExternalInput")
    bq_d = nc.dram_tensor("bq", [DQ], bf16, kind="ExternalInput")
    bk_d = nc.dram_tensor("bk", [DQ], bf16, kind="ExternalInput")
    bv_d = nc.dram_tensor("bv", [DQ], bf16, kind="ExternalInput")
    bo_d = nc.dram_tensor("bo", [D], bf16, kind="ExternalInput")
    out_d = nc.dram_tensor("out", [L, D], f32, kind="ExternalOutput")

    xT_v = xT_d.ap().rearrange("(sc p) (ko s) -> sc p ko s", p=P, ko=KO)
    Wq_v = Wq_d.ap().rearrange("(np p) (ko c) -> np p ko c", p=P, ko=KO)
    Wk_v = Wk_d.ap().rearrange("(np p) (ko c) -> np p ko c", p=P, ko=KO)
    Wv_v = Wv_d.ap().rearrange("p (ko d) -> p ko d", ko=KO)
    Wo_v = Wo_d.ap().rearrange("p (ko e) -> p ko e", ko=WKO)
    out_v = out_d.ap().rearrange("(ms p) e -> p ms e", p=P)

    with tile.TileContext(nc) as tc:
        with (
            tc.tile_pool(name="pp", bufs=1) as pp,
            tc.tile_pool(name="wp", bufs=1) as wp,
            tc.tile_pool(name="sp", bufs=1) as sp,
            tc.tile_pool(name="ps", bufs=1, space="PSUM") as ps,
        ):
            # ---- persistent tiles ----
            KTP = pp.tile([P, NP, L], bf16, name="KTP")
            QTP = pp.tile([P, NP, L], bf16, name="QTP")
            VA = pp.tile([P, ST, HPC, Dh + 1], bf16, name="VA")
            OT = pp.tile([P, WKO, L], bf16, name="OT")
            ones_f = pp.tile([P, P], f32, name="ones_f")
            ones_r = pp.tile([P, P], f32r, name="ones_r")
            ones_b = pp.tile([1, 512], bf16, name="ones_b")
            nc.vector.memset(ones_f[:], 1.0)
            nc.vector.tensor_copy(ones_r[:], ones_f[:])
            nc.vector.memset(ones_b[:], 1.0)
            nc.vector.tensor_copy(VA[:, :, :, Dh:Dh + 1],
                                  ones_f[:, 0:1].to_broadcast((P, ST, HPC, 1)))
            if WB:
                bqs = pp.tile([1, DQ], bf16, name="bqs")
                bks = pp.tile([1, DQ], bf16, name="bks")
                bvs = pp.tile([1, DQ], bf16, name="bvs")
                bos = pp.tile([1, D], bf16, name="bos")
                nc.sync.dma_start(bqs[:], bq_d.ap()[None, :])
                nc.sync.dma_start(bks[:], bk_d.ap()[None, :])
                nc.sync.dma_start(bvs[:], bv_d.ap()[None, :])
                nc.sync.dma_start(bos[:], bo_d.ap()[None, :])

            xts_tiles = [None] * NSC

            def issue_xts_dma(sc):
                xts = sp.tile([P, KO, 512], bf16, tag="xts", bufs=NSC,
                              name=f"xts{sc}")
                nc.sync.dma_start(xts[:], xT_v[sc])
                xts_tiles[sc] = xts

            # ---- projection chains (single-matmul generator steps) ----
            def v_steps(st):
                """V projection for one 128-row seq tile -> VA[:, st]."""
                sc, ssub = st // 4, st % 4
                pv = ps.tile([P, 512], f32, tag="proj", bufs=2,
                             name=f"pv{st}")
                for ko in range(KO):
                    nc.tensor.matmul(
                        pv[:, 0:DQ],
                        lhsT=xts_tiles[sc][:, ko, ssub * P:(ssub + 1) * P],
                        rhs=Wv_sb[:, ko, :],
                        start=(ko == 0), stop=(not WB and ko == KO - 1))
                    yield
                if WB:
                    nc.tensor.matmul(pv[:, 0:DQ], lhsT=ones_b[0:1, 0:P],
                                     rhs=bvs[0:1, :], start=False, stop=True)
                nc.vector.tensor_copy(
                    VA[:, st, :, 0:Dh],
                    pv[:, 0:DQ].rearrange("p (h d) -> p h d", d=Dh))
                yield

            def kt_steps(p, sc, wt_cell):
                """K^T projection: KTP[:, p, sc*512:(sc+1)*512]."""
                pt = ps.tile([P, 512], f32, tag="proj", bufs=2,
                             name=f"pk{sc}_{p}")
                for ko in range(KO):
                    nc.tensor.matmul(pt[:], lhsT=wt_cell[0][:, ko, :],
                                     rhs=xts_tiles[sc][:, ko, :],
                                     start=(ko == 0),
                                     stop=(not WB and ko == KO - 1))
                    yield
                if WB:
                    nc.tensor.matmul(
                        pt[:], lhsT=bks[0:1, p * P:(p + 1) * P],
                        rhs=ones_b[0:1, 0:512], start=False, stop=True)
                nc.vector.tensor_copy(KTP[:, p, sc * 512:(sc + 1) * 512],
                                      pt[:])
                yield

            def qt_steps(p, mc, wt_cell):
                """Q^T projection: QTP[:, p, mc*512:(mc+1)*512]."""
                pt = ps.tile([P, 512], f32, tag="proj", bufs=2,
                             name=f"pq{mc}_{p}")
                for ko in range(KO):
                    nc.tensor.matmul(pt[:], lhsT=wt_cell[0][:, ko, :],
                                     rhs=xts_tiles[mc][:, ko, :],
                                     start=(ko == 0),
                                     stop=(not WB and ko == KO - 1))
                    yield
                if WB:
                    nc.tensor.matmul(
                        pt[:], lhsT=bqs[0:1, p * P:(p + 1) * P],
                        rhs=ones_b[0:1, 0:512], start=False, stop=True)
                nc.vector.tensor_copy(QTP[:, p, mc * 512:(mc + 1) * 512],
                                      pt[:])
                yield

            def wk_prep(p, wt_cell):
                wt = sp.tile([P, KO, P], bf16, tag="wk", bufs=4,
                             name=f"wk{p}")
                nc.sync.dma_start(wt[:], Wk_v[p])
                wt_cell[0] = wt

            def wq_prep(p, wt_cell):
                wt = sp.tile([P, KO, P], bf16, tag="wq", bufs=4,
                             name=f"wq{p}")
                nc.sync.dma_start(wt[:], Wq_v[p])
                wt_cell[0] = wt

            # ---- deferred softmax normalization ----
            pending = []

            def emit_norm_tail(item):
                """Broadcast-reciprocal matmul + normalize into OT."""
                dnr, ot, h, mc = item
                bp = ps.tile([P, 512], f32, tag="proj", bufs=2,
                             name=f"bp{h}_{mc}")
                nc.tensor.matmul(bp[0:Dh, :], lhsT=ones_r[0:1, 0:Dh],
                                 rhs=dnr[0:1, :], start=True, stop=True)
                half = Dh * (h % 2)
                nc.vector.tensor_tensor(
                    OT[half:half + Dh, h // 2, mc * 512:(mc + 1) * 512],
                    ot[:], bp[0:Dh, :], mybir.AluOpType.mult)

            def flush_norms(mcm, max_pair=NP - 1):
                due = [it for it in pending
                       if it[3] == mcm and it[2] // 2 <= max_pair]
                for it in due:
                    pending.remove(it)
                    emit_norm_tail(it)

            def outproj_steps(ms, Wo_sb):
                """Full out-projection chain for one 128-row tile."""
                mcm = (ms * P) // 512
                flush_norms(mcm)
                for ec in range(EC):
                    pt = ps.tile([P, 512], f32, tag="proj", bufs=2,
                                 name=f"po{ms}_{ec}")
                    for ko in range(WKO):
                        nc.tensor.matmul(
                            pt[:], lhsT=OT[:, ko, ms * P:(ms + 1) * P],
                            rhs=Wo_sb[:, ko, ec * 512:(ec + 1) * 512],
                            start=(ko == 0),
                            stop=(not WB and ko == WKO - 1))
                        yield
                    if WB:
                        nc.tensor.matmul(pt[:], lhsT=ones_b[0:1, 0:P],
                                         rhs=bos[0:1,
                                                 ec * 512:(ec + 1) * 512],
                                         start=False, stop=True)
                    os_ = sp.tile([P, 512], f32, tag="os", bufs=3,
                                  name=f"os{ms}_{ec}")
                    nc.vector.tensor_copy(os_[:], pt[:])
                    nc.sync.dma_start(out_v[:, ms, ec * 512:(ec + 1) * 512],
                                      os_[:])
                    yield

            # ko-split out-projection for the LAST mc: pairs {0,1} partial
            # accumulates early to SBUF; tail only runs pairs {2,3} + add.
            osacc = {}

            def opj_partial_steps(ms, Wo_sb):
                mcm = (ms * P) // 512
                flush_norms(mcm, max_pair=1)
                for ec in range(EC):
                    pt = ps.tile([P, 512], f32, tag="proj", bufs=2,
                                 name=f"pp{ms}_{ec}")
                    for ko in range(2):
                        nc.tensor.matmul(
                            pt[:], lhsT=OT[:, ko, ms * P:(ms + 1) * P],
                            rhs=Wo_sb[:, ko, ec * 512:(ec + 1) * 512],
                            start=(ko == 0), stop=(ko == 1))
                        yield
                    acc = sp.tile([P, 512], f32, tag="oacc", bufs=8,
                                  name=f"oacc{ms}_{ec}")
                    nc.vector.tensor_copy(acc[:], pt[:])
                    osacc[(ms, ec)] = acc
                    yield

            def opj_final_steps(ms, Wo_sb):
                mcm = (ms * P) // 512
                flush_norms(mcm)
                for ec in range(EC):
                    pt = ps.tile([P, 512], f32, tag="proj", bufs=2,
                                 name=f"pf{ms}_{ec}")
                    for ko in range(2, WKO):
                        nc.tensor.matmul(
                            pt[:], lhsT=OT[:, ko, ms * P:(ms + 1) * P],
                            rhs=Wo_sb[:, ko, ec * 512:(ec + 1) * 512],
                            start=(ko == 2),
                            stop=(not WB and ko == WKO - 1))
                        yield
                    if WB:
                        nc.tensor.matmul(pt[:], lhsT=ones_b[0:1, 0:P],
                                         rhs=bos[0:1,
                                                 ec * 512:(ec + 1) * 512],
                                         start=False, stop=True)
                    os_ = sp.tile([P, 512], f32, tag="os", bufs=3,
                                  name=f"osf{ms}_{ec}")
                    nc.vector.tensor_tensor(os_[:], pt[:],
                                            osacc[(ms, ec)][:],
                                            mybir.AluOpType.add)
                    nc.sync.dma_start(out_v[:, ms, ec * 512:(ec + 1) * 512],
                                      os_[:])
                    yield

            # ---- FIFO filler queue: [avail, deadline, gen, prep] ----
            fq = []

            def prefetch(k):
                """Issue weight DMAs for the next few queued chains."""
                for ent in fq[:3]:
                    if ent[3] is not None:
                        ent[3]()
                        ent[3] = None

            def drain_overdue(k):
                while fq and fq[0][1] <= k:
                    ent = fq.pop(0)
                    if ent[3] is not None:
                        ent[3]()
                    for _ in ent[2]:
                        pass

            def filler_step(k):
                if fq and fq[0][0] <= k:
                    if fq[0][3] is not None:
                        fq[0][3]()
                        fq[0][3] = None
                    try:
                        next(fq[0][2])
                    except StopIteration:
                        fq.pop(0)
                        filler_step(k)

            # ---- attention slot pipeline --------------------------------
            # staggered block order: each pair's first block (its K/Q
            # deadline) arrives progressively, and each mc column finishes
            # evenly spaced so out-projection dribbles instead of piling
            # into the final phase.
            blocks = [(0, 0), (1, 0), (0, 1), (2, 0), (1, 1), (3, 0),
                      (2, 1), (0, 2), (3, 1), (1, 2), (2, 2), (0, 3),
                      (3, 2), (1, 3), (2, 3), (3, 3)]
            bidx = {b: i for i, b in enumerate(blocks)}
            slots = [(mc, p, st) for p, mc in blocks for st in range(ST)]
            NS = len(slots)
            ops = {}

            def emit_S(k):
                """Row-tiled score pair + the slot's single exp ACT."""
                mc, p, st = slots[k]
                drain_overdue(k)
                spt = ps.tile([P, 1024], f32, tag="spt", bufs=2,
                              name=f"spt{k}")
                ksl = slice(st * P, (st + 1) * P)
                qsl = slice(mc * 512, (mc + 1) * 512)
                nc.tensor.matmul(spt[:, 0:512], lhsT=KTP[0:64, p, ksl],
                                 rhs=QTP[0:64, p, qsl],
                                 start=True, stop=True, tile_position=(0, 0))
                nc.tensor.matmul(spt[:, 512:1024], lhsT=KTP[64:128, p, ksl],
                                 rhs=QTP[64:128, p, qsl],
                                 start=True, stop=True, tile_position=(64, 0))
                es = sp.tile([P, 1024], bf16, tag="es", bufs=5,
                             name=f"es{k}")
                nc.scalar.activation(es[:], spt[:], AF.Exp, scale=0.125)
                return es

            def emit_PV(k, es):
                mc, p, st = slots[k]
                if st == 0:
                    ops[(mc, p)] = [
                        ps.tile([P, 512], f32, tag="op", bufs=2,
                                name=f"op{mc}_{p}_{i}") for i in range(2)]
                opA, opB = ops[(mc, p)]
                nc.tensor.matmul(opA[0:Dh + 1, :], lhsT=VA[:, st, 2 * p, :],
                                 rhs=es[:, 0:512],
                                 start=(st == 0), stop=(st == ST - 1))
                nc.tensor.matmul(opB[0:Dh + 1, :],
                                 lhsT=VA[:, st, 2 * p + 1, :],
                                 rhs=es[:, 512:1024],
                                 start=(st == 0), stop=(st == ST - 1))
                if st == ST - 1:
                    block_end(mc, p)

            def block_end(mc, p):
                """Copy O^T + denominators out of PSUM, queue normalization."""
                pair_ops = ops.pop((mc, p))
                dns, ots = [], []
                for i in range(2):
                    op = pair_ops[i]
                    dn = sp.tile([1, 512], f32, tag="dn", bufs=6,
                                 name=f"dn{mc}_{p}_{i}")
                    nc.vector.tensor_copy(dn[:], op[Dh:Dh + 1, :])
                    ot = sp.tile([Dh, 512], f32, tag="ott", bufs=4,
                                 name=f"ot{mc}_{p}_{i}")
                    nc.vector.tensor_copy(ot[:], op[0:Dh, :])
                    dns.append(dn)
                    ots.append(ot)
                for i in range(2):
                    dn, ot = dns[i], ots[i]
                    nc.vector.reciprocal_approx_fast(dn[:], dn[:])
                    dnr = sp.tile([1, 512], f32r, tag="dnr", bufs=6,
                                  name=f"dnr{mc}_{p}_{i}")
                    nc.vector.tensor_copy(dnr[:], dn[:])
                    pending.append((dnr, ot, 2 * p + i, mc))

            # ---- prologue: DMAs ordered for earliest first score ----
            issue_xts_dma(0)
            wk_cells = {p: [None] for p in range(NP)}
            wq_cells = {p: [None] for p in range(NP)}
            wk_prep(0, wk_cells[0])
            wq_prep(0, wq_cells[0])
            Wv_sb = wp.tile([P, KO, DQ], bf16, name="Wv_sb")
            nc.sync.dma_start(Wv_sb[:], Wv_v)
            for sc in range(1, NSC):
                issue_xts_dma(sc)
            for p in range(1, NP):
                wk_prep(p, wk_cells[p])
                wq_prep(p, wq_cells[p])
            Wo_sb = wp.tile([P, WKO, D], bf16, name="Wo_sb")
            nc.sync.dma_start(Wo_sb[:], Wo_v)
            for _ in kt_steps(0, 0, wk_cells[0]):
                pass
            for _ in qt_steps(0, 0, wq_cells[0]):
                pass

            # ---- build filler queue (sorted by deadline; margins so each
            # chain's Vector CAST lands before its consumer slot) ----
            first_blk = {p: min(bidx[(p, mc)] for mc in range(MC))
                         for p in range(NP)}
            ents = []
            for st in range(ST):
                ents.append([0, st + 2, v_steps(st), None])
            for sc in range(1, NSC):
                ents.append([0, max(1, 4 * sc - 3),
                             kt_steps(0, sc, wk_cells[0]), None])
            for mc in range(1, MC):
                ents.append([0, max(1, 16 * bidx[(0, mc)] - 3),
                             qt_steps(0, mc, wq_cells[0]), None])
            for p in range(1, NP):
                for sc in range(NSC):
                    ents.append([0, 16 * first_blk[p] + 4 * sc - 3,
                                 kt_steps(p, sc, wk_cells[p]), None])
                for mc in range(MC):
                    ents.append([0, 16 * bidx[(p, mc)] - 3,
                                 qt_steps(p, mc, wq_cells[p]), None])
            last_mc = blocks[-1][1]
            for ms in range(MS):
                mcm = (ms * P) // 512
                if mcm == last_mc:
                    avail = 16 * (bidx[(1, mcm)] + 1) + 2 + 2 * (ms % 4)
                    ents.append([avail, min(NS, avail + 30),
                                 opj_partial_steps(ms, Wo_sb), None])
                    ents.append([NS, NS, opj_final_steps(ms, Wo_sb), None])
                else:
                    avail = (16 * (bidx[(NP - 1, mcm)] + 1) + 2
                             + 2 * (ms % 4))
                    ents.append([avail, min(NS, avail + 40),
                                 outproj_steps(ms, Wo_sb), None])
            ents.sort(key=lambda e: (e[1], e[0]))
            fq.extend(ents)

            # ---- main pipeline (PV lags its slot by 2 for jitter slack;
            # fillers go FIRST so they run while ACT drains the spt the
            # next S is waiting on — never behind a blocked PV) ----
            es_live = {0: emit_S(0), 1: emit_S(1)}
            for k in range(NS):
                st = slots[k][2]
                if st in (6, 11) and pending:
                    emit_norm_tail(pending.pop(0))
                else:
                    filler_step(k)
                    filler_step(k)
                    if k < 80 or k >= 112:
                        filler_step(k)
                if k + 2 < NS:
                    es_live[k + 2] = emit_S(k + 2)
                emit_PV(k, es_live.pop(k))

            # ---- tail ----
            while pending:
                emit_norm_tail(pending.pop(0))
            while fq:
                ent = fq.pop(0)
                if ent[3] is not None:
                    ent[3]()
                for _ in ent[2]:
                    pass

    nc.compile()
    return nc


# ---------------------------------------------------------------------------

_NC_CACHE = {}


def _get_nc(with_biases=True):
    key = ("nc", with_biases)
    if key not in _NC_CACHE:
        _NC_CACHE[key] = build_nc(WB=with_biases)
    return _NC_CACHE[key]


def _install_ntff_hook():
    """Provide antenv.axon_hooks (absent in this image) so trace=True can
    capture NTFF profiles for timing."""
    if "antenv.axon_hooks" in sys.modules:
        return
    mod = types.ModuleType("antenv.axon_hooks")
    holder = [None]
    mod.set_axon_ntff_profile_hook = lambda hk: holder.__setitem__(0, hk)
    mod.get_axon_ntff_profile_hook = lambda: holder[0]
    sys.modules["antenv.axon_hooks"] = mod
    import antenv

    antenv.axon_hooks = mod
    try:
        from trn_agent_boot.trn_boot import _ntff_profile_via_ctypes

        mod.set_axon_ntff_profile_hook(
            _ntff_profile_via_ctypes("/opt/axon/libaxon_pjrt.so"))
    except Exception:
        pass


def _make_in_maps(x, Wq, bq, Wk, bk, Wv, bv, Wo, bo):
    import ml_dtypes

    NB, L, D = x.shape          # 4, 2048, 1024
    DQ = D // 2                 # head-group width (8 heads x 64)
    P_, KO, NSC, NP, WKO = 128, D // 128, L // 512, DQ // 128, DQ // 128

    def pack_x(xn):             # [L,D] -> [NSC*128, KO*512]
        return (xn.T.reshape(KO, P_, NSC, 512).transpose(2, 1, 0, 3)
                .reshape(NSC * P_, KO * 512))

    def pack_w(w):              # [D, DQ] -> [NP*128, KO*128]
        return (w.reshape(KO, P_, NP, P_).transpose(2, 1, 0, 3)
                .reshape(NP * P_, KO * P_))

    def pack_rows(w, ko, cols):  # [ko*128, cols] -> [128, ko*cols]
        return (w.reshape(ko, P_, cols).transpose(1, 0, 2)
                .reshape(P_, ko * cols))

    in_maps = []
    for c in range(N_CORES):
        n, g = c % 4, c // 4
        sl = slice(g * DQ, (g + 1) * DQ)
        bf = ml_dtypes.bfloat16
        in_maps.append({
            "xTc": np.ascontiguousarray(pack_x(x[n])).astype(bf),
            "Wqc": np.ascontiguousarray(pack_w(Wq[:, sl])).astype(bf),
            "Wkc": np.ascontiguousarray(pack_w(Wk[:, sl])).astype(bf),
            "Wvc": np.ascontiguousarray(
                pack_rows(Wv[:, sl], KO, DQ)).astype(bf),
            "Woc": np.ascontiguousarray(
                pack_rows(Wo[sl, :], WKO, D)).astype(bf),
            "bq": np.ascontiguousarray(bq[sl]).astype(bf),
            "bk": np.ascontiguousarray(bk[sl]).astype(bf),
            "bv": np.ascontiguousarray(bv[sl]).astype(bf),
            "bo": (bo if g == 0 else np.zeros_like(bo)).astype(bf),
        })
    return in_maps


def run_sharded(inputs, trace=False):
    """Run the SPMD kernel on the full inputs. Returns (output, exec_ns)."""
    wb = any(
        np.asarray(inputs[k]).any() for k in ("bq", "bk", "bv", "bo"))
    nc = _get_nc(with_biases=bool(wb))
    if trace:
        _install_ntff_hook()
    in_maps = _make_in_maps(**inputs)
    res = run_bass_kernel_spmd(nc, in_maps, list(range(N_CORES)), trace=trace)
    outs = [res.results[c]["out"] for c in range(N_CORES)]
    full = np.stack([outs[n] + outs[n + 4] for n in range(4)], axis=0)
    return full.astype(np.float32), res.exec_time_ns


def kernel(**inputs):
    out, _ = run_sharded(inputs, trace=False)
    return out


# revision 44
# speedup vs baseline: 1.2007x; 1.0122x over previous
"""Multi-head attention layer on 8 Trainium2 NeuronCores.

Reference (per batch n):
    Q = x@Wq + bq; K = x@Wk + bk; V = x@Wv + bv       (16 heads, Dh=64)
    out = softmax(Q K^T / sqrt(Dh)) V  -> concat heads -> @Wo + bo

Sharding: 2 head-groups (tensor parallel) x 4 batches (data parallel) = 8
cores. Core c handles batch c%4 and heads [8*(c//4), 8*(c//4)+8). Each core
computes a partial output projection with its Wo row-block; the host sums
the two head-group partials per batch (the only cross-core reduction).

Per-core kernel (bf16 matmul inputs, fp32 PSUM accumulation). The exp
chain on ScalarE (~294us at 1 elem/cycle/lane) is the critical resource;
everything else is scheduled under it:
  - K^T/Q^T in PAIR layout [128, 4, seq]: partitions 0:64 = even head's
    d_head, 64:128 = odd head's. The QK^T scores for the two heads of a
    pair run CONCURRENTLY as 64x128 row-tiles of the PE array
    (tile_position (0,0)/(64,0)), halving score-matmul time vs a padded
    128-row contraction. V in [seq, d_head] layout with an appended ones
    column (softmax denominators fall out of the PV matmul for free).
  - slot pipeline over (mc, pair, st): each slot's two score matmuls fill
    one [128,1024] PSUM tile (512 cols/head); ONE ScalarE exp per slot
    (max-width ACT amortizes its ~352-cycle fixed cost); two PV matmuls
    accumulate per-head O^T (+denominator row). PSUM: spt ping-pong 4
    banks + 2 op accumulators + 1 projection + 1 norm broadcast = 8.
  - projections/out-projection dribble one matmul at a time into PE slack
    between attention matmuls (deadline-ordered FIFO filler queue);
    normalization (broadcast-reciprocal matmul + DVE mult) is deferred
    off the critical path.

Self-contained: hardcodes shapes for x:[4,2048,1024], d_model=1024,
16 heads; a no-bias graph variant is compiled when all biases are zero.
"""

import sys
import types

import numpy as np

import concourse.mybir as mybir
import concourse.tile as tile
from concourse import bacc
from concourse.bass_utils import run_bass_kernel_spmd

f32 = mybir.dt.float32
f32r = mybir.dt.float32r
bf16 = mybir.dt.bfloat16
AF = mybir.ActivationFunctionType
N_CORES = 8
P = 128

# ---------------------------------------------------------------------------


def build_nc(L=2048, D=1024, HPC=8, Dh=64, WB=True):
    """Build the per-core Bass graph (SPMD: same graph, per-core shards)."""
    KO = D // P          # k-tiles over d_model
    DQ = HPC * Dh        # local projected dim (512)
    NP = HPC // 2        # head pairs (4)
    ST = L // P          # 128-row seq (kpos) tiles (16)
    MC = L // 512        # 512-wide query chunks (4)
    NSC = L // 512       # 512-wide seq chunks for projections (4)
    WKO = DQ // P        # k-tiles for out-proj contraction (4)
    EC = D // 512        # 512-wide out chunks (2)
    MS = L // P          # 128-row out row-tiles (16)

    nc = bacc.Bacc("TRN2", target_bir_lowering=False, debug=False,
                   num_devices=N_CORES)

    # host-packed layouts: each partition reads one contiguous line per DMA
    xT_d = nc.dram_tensor("xTc", [NSC * P, KO * 512], bf16,
                          kind="ExternalInput")
    Wq_d = nc.dram_tensor("Wqc", [NP * P, KO * P], bf16,
                          kind="ExternalInput")
    Wk_d = nc.dram_tensor("Wkc", [NP * P, KO * P], bf16,
                          kind="ExternalInput")
    Wv_d = nc.dram_tensor("Wvc", [P, KO * DQ], bf16, kind="ExternalInput")
    Wo_d = nc.dram_tensor("Woc", [P, WKO * D], bf16, kind="ExternalInput")
    bq_d = nc.dram_tensor("bq", [DQ], bf16, kind="ExternalInput")
    bk_d = nc.dram_tensor("bk", [DQ], bf16, kind="ExternalInput")
    bv_d = nc.dram_tensor("bv", [DQ], bf16, kind="ExternalInput")
    bo_d = nc.dram_tensor("bo", [D], bf16, kind="ExternalInput")
    out_d = nc.dram_tensor("out", [L, D], f32, kind="ExternalOutput")

    xT_v = xT_d.ap().rearrange("(sc p) (ko s) -> sc p ko s", p=P, ko=KO)
    Wq_v = Wq_d.ap().rearrange("(np p) (ko c) -> np p ko c", p=P, ko=KO)
    Wk_v = Wk_d.ap().rearrange("(np p) (ko c) -> np p ko c", p=P, ko=KO)
    Wv_v = Wv_d.ap().rearrange("p (ko d) -> p ko d", ko=KO)
    Wo_v = Wo_d.ap().rearrange("p (ko e) -> p ko e", ko=WKO)
    out_v = out_d.ap().rearrange("(ms p) e -> p ms e", p=P)

    with tile.TileContext(nc) as tc:
        with (
            tc.tile_pool(name="pp", bufs=1) as pp,
            tc.tile_pool(name="wp", bufs=1) as wp,
            tc.tile_pool(name="sp", bufs=1) as sp,
            tc.tile_pool(name="ps", bufs=1, space="PSUM") as ps,
        ):
            # ---- persistent tiles ----
            KTP = pp.tile([P, NP, L], bf16, name="KTP")
            QTP = pp.tile([P, NP, L], bf16, name="QTP")
            VA = pp.tile([P, ST, HPC, Dh + 1], bf16, name="VA")
            OT = pp.tile([P, WKO, L], bf16, name="OT")
            ones_f = pp.tile([P, P], f32, name="ones_f")
            ones_r = pp.tile([P, P], f32r, name="ones_r")
            ones_b = pp.tile([1, 512], bf16, name="ones_b")
            nc.vector.memset(ones_f[:], 1.0)
            nc.vector.tensor_copy(ones_r[:], ones_f[:])
            nc.vector.memset(ones_b[:], 1.0)
            nc.vector.tensor_copy(VA[:, :, :, Dh:Dh + 1],
                                  ones_f[:, 0:1].to_broadcast((P, ST, HPC, 1)))
            if WB:
                bqs = pp.tile([1, DQ], bf16, name="bqs")
                bks = pp.tile([1, DQ], bf16, name="bks")
                bvs = pp.tile([1, DQ], bf16, name="bvs")
                bos = pp.tile([1, D], bf16, name="bos")
                nc.sync.dma_start(bqs[:], bq_d.ap()[None, :])
                nc.sync.dma_start(bks[:], bk_d.ap()[None, :])
                nc.sync.dma_start(bvs[:], bv_d.ap()[None, :])
                nc.sync.dma_start(bos[:], bo_d.ap()[None, :])

            xts_tiles = [None] * NSC

            def issue_xts_dma(sc):
                xts = sp.tile([P, KO, 512], bf16, tag="xts", bufs=NSC,
                              name=f"xts{sc}")
                nc.sync.dma_start(xts[:], xT_v[sc])
                xts_tiles[sc] = xts

            # ---- projection chains (single-matmul generator steps) ----
            def v_steps(st):
                """V projection for one 128-row seq tile -> VA[:, st]."""
                sc, ssub = st // 4, st % 4
                pv = ps.tile([P, 512], f32, tag="proj", bufs=2,
                             name=f"pv{st}")
                for ko in range(KO):
                    nc.tensor.matmul(
                        pv[:, 0:DQ],
                        lhsT=xts_tiles[sc][:, ko, ssub * P:(ssub + 1) * P],
                        rhs=Wv_sb[:, ko, :],
                        start=(ko == 0), stop=(not WB and ko == KO - 1))
                    yield
                if WB:
                    nc.tensor.matmul(pv[:, 0:DQ], lhsT=ones_b[0:1, 0:P],
                                     rhs=bvs[0:1, :], start=False, stop=True)
                nc.vector.tensor_copy(
                    VA[:, st, :, 0:Dh],
                    pv[:, 0:DQ].rearrange("p (h d) -> p h d", d=Dh))
                yield

            def kt_steps(p, sc, wt_cell):
                """K^T projection: KTP[:, p, sc*512:(sc+1)*512]."""
                pt = ps.tile([P, 512], f32, tag="proj", bufs=2,
                             name=f"pk{sc}_{p}")
                for ko in range(KO):
                    nc.tensor.matmul(pt[:], lhsT=wt_cell[0][:, ko, :],
                                     rhs=xts_tiles[sc][:, ko, :],
                                     start=(ko == 0),
                                     stop=(not WB and ko == KO - 1))
                    yield
                if WB:
                    nc.tensor.matmul(
                        pt[:], lhsT=bks[0:1, p * P:(p + 1) * P],
                        rhs=ones_b[0:1, 0:512], start=False, stop=True)
                nc.vector.tensor_copy(KTP[:, p, sc * 512:(sc + 1) * 512],
                                      pt[:])
                yield

            def qt_steps(p, mc, wt_cell):
                """Q^T projection: QTP[:, p, mc*512:(mc+1)*512]."""
                pt = ps.tile([P, 512], f32, tag="proj", bufs=2,
                             name=f"pq{mc}_{p}")
                for ko in range(KO):
                    nc.tensor.matmul(pt[:], lhsT=wt_cell[0][:, ko, :],
                                     rhs=xts_tiles[mc][:, ko, :],
                                     start=(ko == 0),
                                     stop=(not WB and ko == KO - 1))
                    yield
                if WB:
                    nc.tensor.matmul(
                        pt[:], lhsT=bqs[0:1, p * P:(p + 1) * P],
                        rhs=ones_b[0:1, 0:512], start=False, stop=True)
                nc.vector.tensor_copy(QTP[:, p, mc * 512:(mc + 1) * 512],
                                      pt[:])
                yield

            def wk_prep(p, wt_cell):
                wt = sp.tile([P, KO, P], bf16, tag="wk", bufs=4,
                             name=f"wk{p}")
                nc.sync.dma_start(wt[:], Wk_v[p])
                wt_cell[0] = wt

            def wq_prep(p, wt_cell):
                wt = sp.tile([P, KO, P], bf16, tag="wq", bufs=4,
                             name=f"wq{p}")
                nc.sync.dma_start(wt[:], Wq_v[p])
                wt_cell[0] = wt

            # ---- deferred softmax normalization ----
            pending = []

            def emit_norm_tail(item):
                """Broadcast-reciprocal matmul + normalize into OT."""
                dnr, ot, h, mc = item
                bp = ps.tile([P, 512], f32, tag="proj", bufs=2,
                             name=f"bp{h}_{mc}")
                nc.tensor.matmul(bp[0:Dh, :], lhsT=ones_r[0:1, 0:Dh],
                                 rhs=dnr[0:1, :], start=True, stop=True)
                half = Dh * (h % 2)
                nc.vector.tensor_tensor(
                    OT[half:half + Dh, h // 2, mc * 512:(mc + 1) * 512],
                    ot[:], bp[0:Dh, :], mybir.AluOpType.mult)

            def flush_norms(mcm, max_pair=NP - 1):
                due = [it for it in pending
                       if it[3] == mcm and it[2] // 2 <= max_pair]
                for it in due:
                    pending.remove(it)
                    emit_norm_tail(it)

            def outproj_steps(ms, Wo_sb):
                """Full out-projection chain for one 128-row tile."""
                mcm = (ms * P) // 512
                flush_norms(mcm)
                for ec in range(EC):
                    pt = ps.tile([P, 512], f32, tag="proj", bufs=2,
                                 name=f"po{ms}_{ec}")
                    for ko in range(WKO):
                        nc.tensor.matmul(
                            pt[:], lhsT=OT[:, ko, ms * P:(ms + 1) * P],
                            rhs=Wo_sb[:, ko, ec * 512:(ec + 1) * 512],
                            start=(ko == 0),
                            stop=(not WB and ko == WKO - 1))
                        yield
                    if WB:
                        nc.tensor.matmul(pt[:], lhsT=ones_b[0:1, 0:P],
                                         rhs=bos[0:1,
                                                 ec * 512:(ec + 1) * 512],
                                         start=False, stop=True)
                    os_ = sp.tile([P, 512], f32, tag="os", bufs=3,
                                  name=f"os{ms}_{ec}")
                    nc.vector.tensor_copy(os_[:], pt[:])
                    nc.sync.dma_start(out_v[:, ms, ec * 512:(ec + 1) * 512],
                                      os_[:])
                    yield

            # ko-split out-projection for the LAST mc: pairs {0,1} partial
            # accumulates early to SBUF; tail only runs pairs {2,3} + add.
            osacc = {}

            def opj_partial_steps(ms, Wo_sb):
                mcm = (ms * P) // 512
                flush_norms(mcm, max_pair=1)
                for ec in range(EC):
                    pt = ps.tile([P, 512], f32, tag="proj", bufs=2,
                                 name=f"pp{ms}_{ec}")
                    for ko in range(2):
                        nc.tensor.matmul(
                            pt[:], lhsT=OT[:, ko, ms * P:(ms + 1) * P],
                            rhs=Wo_sb[:, ko, ec * 512:(ec + 1) * 512],
                            start=(ko == 0), stop=(ko == 1))
                        yield
                    acc = sp.tile([P, 512], f32, tag="oacc", bufs=8,
                                  name=f"oacc{ms}_{ec}")
                    nc.vector.tensor_copy(acc[:], pt[:])
                    osacc[(ms, ec)] = acc
                    yield

            def opj_final_steps(ms, Wo_sb):
                mcm = (ms * P) // 512
                flush_norms(mcm)
                for ec in range(EC):
                    pt = ps.tile([P, 512], f32, tag="proj", bufs=2,
                                 name=f"pf{ms}_{ec}")
                    for ko in range(2, WKO):
                        nc.tensor.matmul(
                            pt[:], lhsT=OT[:, ko, ms * P:(ms + 1) * P],
                            rhs=Wo_sb[:, ko, ec * 512:(ec + 1) * 512],
                            start=(ko == 2),
                            stop=(not WB and ko == WKO - 1))
                        yield
                    if WB:
                        nc.tensor.matmul(pt[:], lhsT=ones_b[0:1, 0:P],
                                         rhs=bos[0:1,
                                                 ec * 512:(ec + 1) * 512],
                                         start=False, stop=True)
                    os_ = sp.tile([P, 512], f32, tag="os", bufs=3,
                                  name=f"osf{ms}_{ec}")
                    nc.vector.tensor_tensor(os_[:], pt[:],
                                            osacc[(ms, ec)][:],
                                            mybir.AluOpType.add)
                    nc.sync.dma_start(out_v[:, ms, ec * 512:(ec + 1) * 512],
                                      os_[:])
                    yield

            # ---- FIFO filler queue: [avail, deadline, gen, prep] ----
            fq = []

            def prefetch(k):
                """Issue weight DMAs for the next few queued chains."""
                for ent in fq[:3]:
                    if ent[3] is not None:
                        ent[3]()
                        ent[3] = None

            def drain_overdue(k):
                while fq and fq[0][1] <= k:
                    ent = fq.pop(0)
                    if ent[3] is not None:
                        ent[3]()
                    for _ in ent[2]:
                        pass

            def filler_step(k):
                if fq and fq[0][0] <= k:
                    if fq[0][3] is not None:
                        fq[0][3]()
                        fq[0][3] = None
                    try:
                        next(fq[0][2])
                    except StopIteration:
                        fq.pop(0)
                        filler_step(k)

            # ---- attention slot pipeline --------------------------------
            # staggered block order: each pair's first block (its K/Q
            # deadline) arrives progressively, and each mc column finishes
            # evenly spaced so out-projection dribbles instead of piling
            # into the final phase.
            blocks = [(0, 0), (1, 0), (0, 1), (2, 0), (1, 1), (3, 0),
                      (2, 1), (0, 2), (3, 1), (1, 2), (2, 2), (0, 3),
                      (3, 2), (1, 3), (2, 3), (3, 3)]
            bidx = {b: i for i, b in enumerate(blocks)}
            slots = [(mc, p, st) for p, mc in blocks for st in range(ST)]
            NS = len(slots)
            ops = {}

            def emit_S(k):
                """Row-tiled score pair + the slot's single exp ACT."""
                mc, p, st = slots[k]
                drain_overdue(k)
                spt = ps.tile([P, 1024], f32, tag="spt", bufs=2,
                              name=f"spt{k}")
                ksl = slice(st * P, (st + 1) * P)
                qsl = slice(mc * 512, (mc + 1) * 512)
                nc.tensor.matmul(spt[:, 0:512], lhsT=KTP[0:64, p, ksl],
                                 rhs=QTP[0:64, p, qsl],
                                 start=True, stop=True, tile_position=(0, 0))
                nc.tensor.matmul(spt[:, 512:1024], lhsT=KTP[64:128, p, ksl],
                                 rhs=QTP[64:128, p, qsl],
                                 start=True, stop=True, tile_position=(64, 0))
                es = sp.tile([P, 1024], bf16, tag="es", bufs=8,
                             name=f"es{k}")
                nc.scalar.activation(es[:], spt[:], AF.Exp, scale=0.125)
                return es

            def emit_PV(k, es):
                mc, p, st = slots[k]
                if st == 0:
                    ops[(mc, p)] = [
                        ps.tile([P, 512], f32, tag="op", bufs=2,
                                name=f"op{mc}_{p}_{i}") for i in range(2)]
                opA, opB = ops[(mc, p)]
                nc.tensor.matmul(opA[0:Dh + 1, :], lhsT=VA[:, st, 2 * p, :],
                                 rhs=es[:, 0:512],
                                 start=(st == 0), stop=(st == ST - 1))
                nc.tensor.matmul(opB[0:Dh + 1, :],
                                 lhsT=VA[:, st, 2 * p + 1, :],
                                 rhs=es[:, 512:1024],
                                 start=(st == 0), stop=(st == ST - 1))
                if st == ST - 1:
                    block_end(mc, p)

            def block_end(mc, p):
                """Copy O^T + denominators out of PSUM, queue normalization."""
                pair_ops = ops.pop((mc, p))
                dns, ots = [], []
                for i in range(2):
                    op = pair_ops[i]
                    dn = sp.tile([1, 512], f32, tag="dn", bufs=6,
                                 name=f"dn{mc}_{p}_{i}")
                    nc.vector.tensor_copy(dn[:], op[Dh:Dh + 1, :])
                    ot = sp.tile([Dh, 512], f32, tag="ott", bufs=4,
                                 name=f"ot{mc}_{p}_{i}")
                    nc.vector.tensor_copy(ot[:], op[0:Dh, :])
                    dns.append(dn)
                    ots.append(ot)
                for i in range(2):
                    dn, ot = dns[i], ots[i]
                    nc.vector.reciprocal_approx_fast(dn[:], dn[:])
                    dnr = sp.tile([1, 512], f32r, tag="dnr", bufs=6,
                                  name=f"dnr{mc}_{p}_{i}")
                    nc.vector.tensor_copy(dnr[:], dn[:])
                    pending.append((dnr, ot, 2 * p + i, mc))

            # ---- prologue: DMAs ordered for earliest first score ----
            issue_xts_dma(0)
            wk_cells = {p: [None] for p in range(NP)}
            wq_cells = {p: [None] for p in range(NP)}
            wk_prep(0, wk_cells[0])
            wq_prep(0, wq_cells[0])
            Wv_sb = wp.tile([P, KO, DQ], bf16, name="Wv_sb")
            nc.sync.dma_start(Wv_sb[:], Wv_v)
            for sc in range(1, NSC):
                issue_xts_dma(sc)
            for p in range(1, NP):
                wk_prep(p, wk_cells[p])
                wq_prep(p, wq_cells[p])
            Wo_sb = wp.tile([P, WKO, D], bf16, name="Wo_sb")
            nc.sync.dma_start(Wo_sb[:], Wo_v)
            for _ in kt_steps(0, 0, wk_cells[0]):
                pass
            for _ in qt_steps(0, 0, wq_cells[0]):
                pass

            # ---- build filler queue (sorted by deadline; margins so each
            # chain's Vector CAST lands before its consumer slot) ----
            first_blk = {p: min(bidx[(p, mc)] for mc in range(MC))
                         for p in range(NP)}
            ents = []
            for st in range(ST):
                ents.append([0, st + 2, v_steps(st), None])
            for sc in range(1, NSC):
                ents.append([0, max(1, 4 * sc - 3),
                             kt_steps(0, sc, wk_cells[0]), None])
            for mc in range(1, MC):
                ents.append([0, max(1, 16 * bidx[(0, mc)] - 3),
                             qt_steps(0, mc, wq_cells[0]), None])
            for p in range(1, NP):
                for sc in range(NSC):
                    ents.append([0, 16 * first_blk[p] + 4 * sc - 3,
                                 kt_steps(p, sc, wk_cells[p]), None])
                for mc in range(MC):
                    ents.append([0, 16 * bidx[(p, mc)] - 3,
                                 qt_steps(p, mc, wq_cells[p]), None])
            last_mc = blocks[-1][1]
            for ms in range(MS):
                mcm = (ms * P) // 512
                if mcm == last_mc:
                    avail = 16 * (bidx[(1, mcm)] + 1) + 2 + 2 * (ms % 4)
                    ents.append([avail, min(NS - 4, avail + 30),
                                 opj_partial_steps(ms, Wo_sb), None])
                    ents.append([NS + 8, NS + 8,
                                 opj_final_steps(ms, Wo_sb), None])
                else:
                    avail = (16 * (bidx[(NP - 1, mcm)] + 1) + 2
                             + 2 * (ms % 4))
                    ents.append([avail, min(NS, avail + 40),
                                 outproj_steps(ms, Wo_sb), None])
            ents.sort(key=lambda e: (e[1], e[0]))
            fq.extend(ents)

            # ---- main pipeline (PV lags its slot by 4 so late V/norm
            # Vector work never blocks the S/ACT chain queued behind it;
            # fillers go FIRST so they run while ACT drains the spt the
            # next S is waiting on) ----
            LAG = 4
            es_live = {j: emit_S(j) for j in range(LAG - 2)}
            for k in range(NS + LAG - 2):
                kS = k + 2
                kP = k - (LAG - 2)
                st = slots[min(k, NS - 1)][2]
                if st in (6, 11) and pending:
                    emit_norm_tail(pending.pop(0))
                else:
                    filler_step(k)
                    filler_step(k)
                    if k < 80 or k >= 112:
                        filler_step(k)
                if kS < NS:
                    es_live[kS] = emit_S(kS)
                if 0 <= kP < NS:
                    emit_PV(kP, es_live.pop(kP))

            # ---- tail ----
            while pending:
                emit_norm_tail(pending.pop(0))
            while fq:
                ent = fq.pop(0)
                if ent[3] is not None:
                    ent[3]()
                for _ in ent[2]:
                    pass

    nc.compile()
    return nc


# ---------------------------------------------------------------------------

_NC_CACHE = {}


def _get_nc(with_biases=True):
    key = ("nc", with_biases)
    if key not in _NC_CACHE:
        _NC_CACHE[key] = build_nc(WB=with_biases)
    return _NC_CACHE[key]


def _install_ntff_hook():
    """Provide antenv.axon_hooks (absent in this image) so trace=True can
    capture NTFF profiles for timing."""
    if "antenv.axon_hooks" in sys.modules:
        return
    mod = types.ModuleType("antenv.axon_hooks")
    holder = [None]
    mod.set_axon_ntff_profile_hook = lambda hk: holder.__setitem__(0, hk)
    mod.get_axon_ntff_profile_hook = lambda: holder[0]
    sys.modules["antenv.axon_hooks"] = mod
    import antenv

    antenv.axon_hooks = mod
    try:
        from trn_agent_boot.trn_boot import _ntff_profile_via_ctypes

        mod.set_axon_ntff_profile_hook(
            _ntff_profile_via_ctypes("/opt/axon/libaxon_pjrt.so"))
    except Exception:
        pass


def _make_in_maps(x, Wq, bq, Wk, bk, Wv, bv, Wo, bo):
    import ml_dtypes

    NB, L, D = x.shape          # 4, 2048, 1024
    DQ = D // 2                 # head-group width (8 heads x 64)
    P_, KO, NSC, NP, WKO = 128, D // 128, L // 512, DQ // 128, DQ // 128

    def pack_x(xn):             # [L,D] -> [NSC*128, KO*512]
        return (xn.T.reshape(KO, P_, NSC, 512).transpose(2, 1, 0, 3)
                .reshape(NSC * P_, KO * 512))

    def pack_w(w):              # [D, DQ] -> [NP*128, KO*128]
        return (w.reshape(KO, P_, NP, P_).transpose(2, 1, 0, 3)
                .reshape(NP * P_, KO * P_))

    def pack_rows(w, ko, cols):  # [ko*128, cols] -> [128, ko*cols]
        return (w.reshape(ko, P_, cols).transpose(1, 0, 2)
                .reshape(P_, ko * cols))

    in_maps = []
    for c in range(N_CORES):
        n, g = c % 4, c // 4
        sl = slice(g * DQ, (g + 1) * DQ)
        bf = ml_dtypes.bfloat16
        in_maps.append({
            "xTc": np.ascontiguousarray(pack_x(x[n])).astype(bf),
            "Wqc": np.ascontiguousarray(pack_w(Wq[:, sl])).astype(bf),
            "Wkc": np.ascontiguousarray(pack_w(Wk[:, sl])).astype(bf),
            "Wvc": np.ascontiguousarray(
                pack_rows(Wv[:, sl], KO, DQ)).astype(bf),
            "Woc": np.ascontiguousarray(
                pack_rows(Wo[sl, :], WKO, D)).astype(bf),
            "bq": np.ascontiguousarray(bq[sl]).astype(bf),
            "bk": np.ascontiguousarray(bk[sl]).astype(bf),
            "bv": np.ascontiguousarray(bv[sl]).astype(bf),
            "bo": (bo if g == 0 else np.zeros_like(bo)).astype(bf),
        })
    return in_maps


def run_sharded(inputs, trace=False):
    """Run the SPMD kernel on the full inputs. Returns (output, exec_ns)."""
    wb = any(
        np.asarray(inputs[k]).any() for k in ("bq", "bk", "bv", "bo"))
    nc = _get_nc(with_biases=bool(wb))
    if trace:
        _install_ntff_hook()
    in_maps = _make_in_maps(**inputs)
    res = run_bass_kernel_spmd(nc, in_maps, list(range(N_CORES)), trace=trace)
    outs = [res.results[c]["out"] for c in range(N_CORES)]
    full = np.stack([outs[n] + outs[n + 4] for n in range(4)], axis=0)
    return full.astype(np.float32), res.exec_time_ns


def kernel(**inputs):
    out, _ = run_sharded(inputs, trace=False)
    return out
